# revision 34
# baseline (speedup 1.0000x reference)
"""Grok1-style attention on 8 trn2 NeuronCores, tensor-parallel over heads.

Sharding (per core c of 8):
  - q heads 4c..4c+3 (512 features), kv head c (128+128 features)
  - w_qkv sharded column-wise (by head), w_o row-wise; partial o_proj
    outputs summed on host (the all-reduce).

Device layout: qkv computed TRANSPOSED (features on partitions, positions
on free axis) so scores (k^T q), probs*V and o_proj chain without
transposes except 16 cheap PE transposes of V.

Perf structure:
  - all inputs host-cast to bf16, loads split across 3 DMA rings
    (gpsimd / sync / scalar) so the first qkv matmul starts early.
  - PSUM used as four 2-bank PAIR tiles: "sc" ring x2 (scores/o_proj/
    qkv) + "acc" ring x2 ([attn | denom] per head).  One tanh and one
    exp instruction covers a 2-bank score pair -> half the ACT
    instruction overhead, and the ACT->PE chain has a 2-pair runway.
  - causal mask is ADDITIVE: an identity@mask matmul opens the PSUM
    accumulation group with -3000 on masked elements, the score matmul
    accumulates onto it (same PE group -> no cross-engine ordering
    hazard); tanh saturates to -1, exp gives e^-30 ~ 1e-13, so there is
    no mask multiply in the critical chain at all.
  - denominator rows via an all-ones [128,128] lhsT (broadcast row-sum
    on all PSUM partitions), reciprocal_approx_fast (~0.65us, 18-bit)
    instead of the 3.3us exact reciprocal.
  - o_proj(qt-1) pairs interleaved into qt's attention as PE filler
    (qt0 instead round-robins two heads to hide the ACT chain); bf16
    partial outputs written as 1024-col pairs split across two DMA
    rings; host sums partials in fp64.
"""
import numpy as np
from contextlib import ExitStack

import concourse.bass as bass
import concourse.mybir as mybir
import concourse.tile as tile
from concourse import bacc
from concourse.bass_utils import run_bass_kernel_spmd
from concourse.masks import make_identity

T = 2048
D = 4096
HD = 128
HALF = 64
NCORES = 8
HPC = 4                    # q heads per core
QF = HPC * HD              # 512
NF = QF + 2 * HD           # 768 qkv features per core
NCH = D // 128             # 32 contraction chunks
TT = 512                   # t-tile width (matmul moving dim)
NTT = T // TT              # 4
NKT = T // 128             # 16 k-tiles
SCALING = HD ** -0.5
CAP = 30.0
MASKNEG = -3000.0
BF = mybir.dt.bfloat16
F32 = mybir.dt.float32
BF_NP = mybir.dt.np(BF)


def _emit(nc):
    hS = nc.dram_tensor("hS", [128, 32, 4, TT], BF, kind="ExternalInput").ap()
    wqS = nc.dram_tensor("wqS", [128, 16, 2, NF], BF, kind="ExternalInput").ap()
    woS = nc.dram_tensor("woS", [128, 2, 2, D], BF, kind="ExternalInput").ap()
    cc = nc.dram_tensor("cc", [HD, T], BF, kind="ExternalInput").ap()
    ss = nc.dram_tensor("ss", [HD, T], BF, kind="ExternalInput").ap()
    mg = nc.dram_tensor("mg", [128, 4, TT], BF, kind="ExternalInput").ap()
    out = nc.dram_tensor("out", [T, D], BF, kind="ExternalOutput").ap()

    with tile.TileContext(nc) as tc:
        with ExitStack() as ctx:
            wqp = ctx.enter_context(tc.tile_pool(name="wqp", bufs=1))
            hp = ctx.enter_context(tc.tile_pool(name="hp", bufs=10))
            cstp = ctx.enter_context(tc.tile_pool(name="cstp", bufs=1))
            seqp = ctx.enter_context(tc.tile_pool(name="seqp", bufs=1))
            rtp = ctx.enter_context(tc.tile_pool(name="rtp", bufs=2))
            etp = ctx.enter_context(tc.tile_pool(name="etp", bufs=4))
            smp = ctx.enter_context(tc.tile_pool(name="smp", bufs=2))
            obp = ctx.enter_context(tc.tile_pool(name="obp", bufs=3))
            psp = ctx.enter_context(tc.tile_pool(name="psp", bufs=2, space="PSUM"))

            # ---- resident loads ----
            # gpsimd ring: 16 small wq chunks (critical path) then rot DMAs;
            # sync ring: h even pieces then wo; scalar ring: h odd pieces
            # then cc/ss/mneg.
            wq_t = []
            for g in range(16):
                w_g = wqp.tile([128, 2, NF], BF, tag=f"wq{g}", name=f"wq{g}")
                eng = nc.gpsimd if g % 2 == 0 else nc.sync
                if g == 0:
                    eng.dma_start(w_g[:, 0:1, :], wqS[:, g, 0:1, :])
                    eng.dma_start(w_g[:, 1:2, :], wqS[:, g, 1:2, :])
                else:
                    eng.dma_start(w_g[:], wqS[:, g, :, :])
                wq_t.append(w_g)
            cc_sb = cstp.tile([HD, T], BF, tag="cc")
            ss_sb = cstp.tile([HD, T], BF, tag="ss")
            mneg = cstp.tile([128, 4, TT], BF, tag="mneg")
            wo_t = [wqp.tile([128, 2, D], BF, tag=f"wo{j}", name=f"wo{j}")
                    for j in range(2)]
            ident = cstp.tile([128, 128], BF, tag="id")
            make_identity(nc, ident[:])
            ones_kk = cstp.tile([128, 128], BF, tag="ones_kk")
            nc.gpsimd.memset(ones_kk[:], 1.0)

            qTt = [[seqp.tile([HD, TT], BF, tag=f"q{h}_{tt}", name=f"qT{h}_{tt}")
                    for tt in range(NTT)] for h in range(HPC)]
            kTt = [seqp.tile([HD, TT], BF, tag=f"k_{tt}", name=f"kT{tt}")
                   for tt in range(NTT)]
            vTt = [seqp.tile([HD, TT], BF, tag=f"v_{tt}", name=f"vT{tt}")
                   for tt in range(NTT)]
            vbt = [seqp.tile([128, HD], BF, tag=f"vb_{kt}", name=f"vb{kt}")
                   for kt in range(NKT)]
            atq = [[seqp.tile([HD, TT], BF, tag=f"a{h}_{qt}", name=f"at{h}_{qt}")
                    for qt in range(NTT)] for h in range(HPC)]

            # ---- phase 1: qkv projection (transposed) + rope ----
            for tt in range(NTT):
                t0 = tt * TT
                h_t = []
                for i in range(8):
                    h_i = hp.tile([128, 4, TT], BF, tag="h", name=f"h{tt}_{i}")
                    if tt == 0:
                        eng = nc.scalar
                    else:
                        eng = nc.sync if i % 2 == 0 else nc.scalar
                    eng.dma_start(h_i[:], hS[:, 8 * tt + i, :, :])
                    h_t.append(h_i)
                if tt == 0:
                    nc.scalar.dma_start(cc_sb[:], cc[:, :])
                    nc.scalar.dma_start(ss_sb[:], ss[:, :])
                    nc.scalar.dma_start(mneg[:], mg[:, :, :])
                if tt == 1:
                    for j in range(2):
                        nc.sync.dma_start(wo_t[j][:], woS[:, j, :, :])
                c_t = cc_sb[:, t0:t0 + TT]
                s_t = ss_sb[:, t0:t0 + TT]
                for fg in range(2):
                    pA = psp.tile([128, 2, TT], F32, tag="sc", name="qkv_A")
                    pB = psp.tile([128, 2, TT], F32, tag="acc", name="qkv_B")
                    tgt = [pA[:, 0, :], pA[:, 1, :], pB[:, 0, :]]
                    for c in range(NCH):
                        src = h_t[c // 4][:, c % 4, :]
                        for j in range(3):
                            f = fg * 3 + j
                            nc.tensor.matmul(
                                tgt[j],
                                wq_t[c // 2][:, c % 2, f * 128:(f + 1) * 128],
                                src,
                                start=(c == 0),
                                stop=(c == NCH - 1),
                            )
                    for j in range(3):
                        f = fg * 3 + j
                        if f < 5:
                            dst = qTt[f][tt] if f < HPC else kTt[tt]
                            qk_sb = rtp.tile([128, TT], BF, tag="qk_sb")
                            if j == 1:
                                nc.vector.tensor_copy(qk_sb[:], tgt[j])
                            else:
                                nc.scalar.copy(qk_sb[:], tgt[j])
                            # rotated copy: [x2; x1] via partition-swap DMA
                            rot = rtp.tile([128, TT], BF, tag="rot")
                            nc.gpsimd.dma_start(rot[0:HALF, :], qk_sb[HALF:128, :])
                            nc.gpsimd.dma_start(rot[HALF:128, :], qk_sb[0:HALF, :])
                            m1 = rtp.tile([128, TT], BF, tag="m1")
                            nc.vector.tensor_mul(m1[:], qk_sb[:], c_t)
                            m2 = rtp.tile([128, TT], BF, tag="m2")
                            nc.vector.tensor_mul(m2[:], rot[:], s_t)
                            nc.vector.tensor_add(dst[:], m1[:], m2[:])
                        else:
                            nc.scalar.copy(vTt[tt][:], tgt[j])
                # transpose this t-tile's V to [t, d] blocks (PSUM bf16)
                tp = psp.tile([128, 4, 128], BF, tag="acc", name="tp")
                for i in range(4):
                    kt = 4 * tt + i
                    nc.tensor.transpose(
                        tp[:, i, :],
                        vTt[tt][:, i * 128:(i + 1) * 128],
                        ident[:])
                    nc.vector.tensor_copy(vbt[kt][:], tp[:, i, :])

            # ---- phase 2+3: attention (paired banks, additive mask via a
            #      same-group PE matmul, 2-pair runway); o_proj(qt-1) pairs
            #      interleaved into qt's attention as PE filler ----
            def emit_score_pair(qt, h, p):
                sp = psp.tile([128, 2, TT], F32, tag="sc", name="s_pair")
                info = []
                for s_i in range(2):
                    kt = 2 * p + s_i
                    m = kt - 4 * qt
                    j0 = 128 * m if m >= 0 else 0
                    if m >= 0:
                        nc.tensor.matmul(
                            sp[:, s_i, j0:TT], ident[:], mneg[:, m, j0:TT],
                            start=True, stop=False,
                        )
                    nc.tensor.matmul(
                        sp[:, s_i, j0:TT],
                        kTt[kt // 4][:, (kt % 4) * 128:(kt % 4 + 1) * 128],
                        qTt[h][qt][:, j0:TT],
                        start=(m < 0), stop=True,
                    )
                    info.append((kt, j0))
                return sp, info

            def emit_o_pair(qt, t16, half, np_, ring=None):
                r0 = t16 * 128
                n0 = (half * 4 + np_ * 2) * TT
                pl = psp.tile([128, 2, TT], F32, tag="sc", name="o_pair")
                for sub in range(2):
                    for fc in range(HPC):
                        lhsT = atq[fc][qt][:, (t16 % 4) * 128:(t16 % 4 + 1) * 128]
                        nc.tensor.matmul(
                            pl[:, sub, :], lhsT,
                            wo_t[fc // 2][:, fc % 2,
                                          n0 + sub * TT:n0 + (sub + 1) * TT],
                            start=(fc == 0), stop=(fc == HPC - 1),
                        )
                ob = obp.tile([128, 2, TT], BF, tag="ob")
                nc.vector.tensor_copy(ob[:], pl[:])
                if ring is None:
                    ring = nc.sync if (t16 + half) % 2 == 0 else nc.gpsimd
                ring.dma_start(
                    out[r0:r0 + 128, n0:n0 + 2 * TT],
                    ob[:].rearrange("p a t -> p (a t)"))

            def o_pair_args(qt):
                return [(qt, t16, half, np_)
                        for t16 in range(4 * qt, 4 * qt + 4)
                        for half in range(2)
                        for np_ in range(2)]

            # qt0: no o_proj fillers exist, so hide each head's tanh/exp
            # chain behind the partner head's score matmuls instead.
            for hp_ in range(2):
                heads = [2 * hp_, 2 * hp_ + 1]
                accs = {}
                cur = {}
                for hx in heads:
                    accs[hx] = psp.tile([HD, 2, TT], F32, tag="acc",
                                        name=f"acc0_{hx}")
                for hx in heads:
                    cur[hx] = emit_score_pair(0, hx, 0)
                for p in range(2):
                    for hx in heads:
                        sp, info = cur[hx]
                        et = etp.tile([128, 2, TT], BF, tag="et")
                        jc = min(j0 for _, j0 in info)
                        nc.scalar.activation(
                            sp[:, :, jc:TT], sp[:, :, jc:TT],
                            mybir.ActivationFunctionType.Tanh,
                            scale=SCALING / CAP,
                        )
                        nc.scalar.activation(
                            et[:, :, jc:TT], sp[:, :, jc:TT],
                            mybir.ActivationFunctionType.Exp,
                            scale=CAP,
                        )
                        for s_i, (kt, j0) in enumerate(info):
                            nc.tensor.matmul(
                                accs[hx][:, 0, j0:TT], vbt[kt][:],
                                et[:, s_i, j0:TT],
                                start=(kt == 0), stop=(kt == 3),
                            )
                            nc.tensor.matmul(
                                accs[hx][:, 1, j0:TT], ones_kk[:],
                                et[:, s_i, j0:TT],
                                start=(kt == 0), stop=(kt == 3),
                            )
                        if p == 0:
                            cur[hx] = emit_score_pair(0, hx, 1)
                for hx in heads:
                    rcp = smp.tile([128, TT], F32, tag="rcp")
                    nc.vector.reciprocal_approx_fast(rcp[:], accs[hx][:, 1, :])
                    nc.vector.tensor_mul(atq[hx][0][:], accs[hx][:, 0, :], rcp[:])

            for qt in range(1, NTT):
                fillers = o_pair_args(qt - 1)[::-1]
                nslots = HPC * (4 * qt + 4) // 2
                slot = 0
                pend_next = None
                for h in range(HPC):
                    acc = psp.tile([HD, 2, TT], F32, tag="acc", name="acc")
                    nkt = 4 * qt + 4
                    NP = nkt // 2
                    if pend_next is not None:
                        pend = pend_next
                        pend_next = None
                    else:
                        pend = [emit_score_pair(qt, h, 0)]
                        if fillers:
                            emit_o_pair(*fillers.pop())
                        if NP > 1:
                            pend.append(emit_score_pair(qt, h, 1))
                    for p in range(NP):
                        sp, info = pend[p]
                        et = etp.tile([128, 2, TT], BF, tag="et")
                        jc = min(j0 for _, j0 in info)
                        nc.scalar.activation(
                            sp[:, :, jc:TT], sp[:, :, jc:TT],
                            mybir.ActivationFunctionType.Tanh,
                            scale=SCALING / CAP,
                        )
                        nc.scalar.activation(
                            et[:, :, jc:TT], sp[:, :, jc:TT],
                            mybir.ActivationFunctionType.Exp,
                            scale=CAP,
                        )
                        for s_i, (kt, j0) in enumerate(info):
                            last = kt == nkt - 1
                            nc.tensor.matmul(
                                acc[:, 0, j0:TT], vbt[kt][:], et[:, s_i, j0:TT],
                                start=(kt == 0), stop=last,
                            )
                            nc.tensor.matmul(
                                acc[:, 1, j0:TT], ones_kk[:], et[:, s_i, j0:TT],
                                start=(kt == 0), stop=last,
                            )
                        if p + 2 < NP:
                            pend.append(emit_score_pair(qt, h, p + 2))
                        elif p == NP - 1 and h + 1 < HPC:
                            # next block's runway fills this block's ACT drain
                            pend_next = [emit_score_pair(qt, h + 1, 0)]
                            if fillers:
                                emit_o_pair(*fillers.pop())
                            pend_next.append(emit_score_pair(qt, h + 1, 1))
                        if fillers and (slot + 1) * 16 // nslots > slot * 16 // nslots:
                            emit_o_pair(*fillers.pop())
                        slot += 1
                    rcp = smp.tile([128, TT], F32, tag="rcp")
                    nc.vector.reciprocal_approx_fast(rcp[:], acc[:, 1, :])
                    nc.vector.tensor_mul(atq[h][qt][:], acc[:, 0, :], rcp[:])
                while fillers:
                    emit_o_pair(*fillers.pop())
            rings = [nc.sync, nc.gpsimd, nc.scalar]
            for i, args in enumerate(o_pair_args(NTT - 1)):
                emit_o_pair(*args, ring=rings[i % 3])
    return nc


_CACHE = {}


def _get_nc():
    if "nc" not in _CACHE:
        nc = bacc.Bacc("TRN2", target_bir_lowering=False, debug=False)
        _emit(nc)
        nc.compile()
        _CACHE["nc"] = nc
    return _CACHE["nc"]


def _in_maps(positions, hidden_states, w_qkv, w_o):
    hidden_states = np.asarray(hidden_states, dtype=np.float32)
    w_qkv = np.asarray(w_qkv, dtype=np.float32)
    w_o = np.asarray(w_o, dtype=np.float32)
    pos = np.asarray(positions).astype(np.float64)

    # hS[p, tt*8+i, c4, t'] = hidden[tt*TT+t', (4i+c4)*128+p]
    hS = np.ascontiguousarray(
        hidden_states.reshape(NTT, TT, NCH, 128).transpose(3, 0, 2, 1)
        .reshape(128, NTT * 8, 4, TT)).astype(BF_NP)
    inv_freq = 1.0 / (10000.0 ** (np.arange(HALF, dtype=np.float64) * 2.0 / HD))
    ang = np.outer(inv_freq, pos)                      # [64, T]
    cos = np.cos(ang).astype(np.float32)
    sin = np.sin(ang).astype(np.float32)
    ccm = np.concatenate([cos, cos], axis=0).astype(BF_NP)   # [128, T]
    ssm = np.concatenate([-sin, sin], axis=0).astype(BF_NP)  # [128, T]
    ii = np.arange(128)[:, None]
    jj = np.arange(TT)[None, :]
    mgm = np.stack([(jj - ii - 128 * m < 0) for m in range(4)])
    mgm = np.ascontiguousarray((mgm * MASKNEG).transpose(1, 0, 2)).astype(BF_NP)

    in_maps = []
    for c in range(NCORES):
        rows = np.concatenate([
            w_qkv[QF * c:QF * (c + 1)],
            w_qkv[D + HD * c:D + HD * (c + 1)],
            w_qkv[D + HD * NCORES + HD * c:D + HD * NCORES + HD * (c + 1)],
        ], axis=0)                                      # [768, 4096]
        wq_c = rows.T                                   # [4096, 768]
        wqS = np.ascontiguousarray(
            wq_c.reshape(NCH, 128, NF).transpose(1, 0, 2)
            .reshape(128, 16, 2, NF)).astype(BF_NP)
        wo_c = w_o[:, QF * c:QF * (c + 1)].T            # [512, 4096]
        woS = np.ascontiguousarray(
            wo_c.reshape(4, 128, D).transpose(1, 0, 2)
            .reshape(128, 2, 2, D)).astype(BF_NP)
        in_maps.append({"hS": hS, "wqS": wqS, "woS": woS,
                        "cc": ccm, "ss": ssm, "mg": mgm})
    return in_maps


def run(positions, hidden_states, w_qkv, w_o, trace=False):
    nc = _get_nc()
    in_maps = _in_maps(positions, hidden_states, w_qkv, w_o)
    res = run_bass_kernel_spmd(nc, in_maps, list(range(NCORES)), trace=trace)
    parts = np.stack([np.asarray(res.results[i]["out"], dtype=np.float64)
                      for i in range(NCORES)], axis=0)
    full = parts.sum(axis=0).astype(np.float32)
    return full, res


def kernel(positions, hidden_states, w_qkv, w_o):
    full, _ = run(positions, hidden_states, w_qkv, w_o, trace=False)
    return full
